# revision 37
# baseline (speedup 1.0000x reference)
"""Multi-head causal attention on 8 Trainium2 NeuronCores.

Problem: resid_pre [4, 2048, 1024], 16 heads x d_head 64, causal softmax,
output [4, 2048, 1024] f32.

Sharding: data-parallel over the 4 batches x tensor-parallel over 2 head
groups (8 heads each) -> 8 cores. Each core computes the attention output
contribution of its 8 heads for its batch; the host sums the two head-group
partials per batch (the "all-reduce") and adds the output bias.

Per-core kernel (matmul inputs bf16, accumulation fp32 in PSUM):

  prelude, pipelined by 512-column blocks of X^T (causality means attention
  superblock sb only needs Q/K columns <= (sb+1)*512):
    V = X @ W_v for all 8 heads in natural [seq, d] layout with a ones
    column appended per head, and Q^T/K^T for head pair 0, pair-stacked on
    partitions (head 2p in partitions 0-63, 2p+1 in 64-127). Warmup
    matmuls on zeroed tiles keep the PE HAM clock-gate at 8/8 through the
    DMA gaps.

  per head pair p (heads 2p, 2p+1), per 512-wide query superblock, per
  128-wide key tile:
    S^T = K^T.T @ Q^T (keys on partitions, one matmul per head via
    partition row groups, concurrent), restricted to the un-masked column
    suffix; causal triangle (-500) added to the diagonal block in-PSUM via
    an identity-stationary matmul. exp is split across TWO engines to
    break the ScalarE pacing: most key tiles exp on ScalarE; ~1/4 use a
    Schraudolph bit-trick on VectorE (i = round(A*s + B) as int16, bits
    reinterpreted as bf16 ~= exp(s/8), +-3% sawtooth that the softmax
    ratio averages out). z~^T[65, 512] += V_chunk.T @ P~^T accumulated in
    PSUM; row 64 (ones column) is the softmax denominator. Normalize
    first evacuates z PSUM to SBUF fp32 (frees the bank for the next
    superblock's PV immediately), then reciprocal_approx_fast + gpsimd
    partition broadcast + multiply off the critical path.
    Pair p+1's Q/K projection matmuls are interleaved into this stream so
    the PE never starves; for the last pair the output projection tiles of
    already-final superblocks are interleaved instead.

  output projection: out[q, m] = sum_p z^T_p.T @ W_o_p, PSUM -> SBUF
  (bf16) -> DRAM. Partials are bf16; the host sums the two head-group
  partials in fp32.

b_Q/b_K are applied on-device (per-partition bias during the PSUM->SBUF
copy); b_V's exact contribution sum_h W_O[h].T @ b_V[h] (softmax rows sum
to 1) and b_O are added on the host.
"""
import math

import ml_dtypes
import numpy as np

import concourse.bass as bass
import concourse.mybir as mybir
import concourse.tile as tile
from concourse import bacc
from concourse import bass_utils

F32 = mybir.dt.float32
BF16 = mybir.dt.bfloat16
I16 = mybir.dt.int16
EXPF = mybir.ActivationFunctionType.Exp

S = 2048          # sequence length
DM = 1024         # d_model
DH = 64           # d_head
NHC = 8           # heads per core
PAIRS = 4         # head pairs per core
MC = 8            # d_model chunks of 128
NSB = 4           # query superblocks of 512
SBW = 512         # superblock width
NKT = 16          # key tiles of 128
NST = 16          # seq tiles of 128
SCALE = 0.125     # 1/sqrt(d_head)

F_SCH = True      # Schraudolph exp on DVE
F_WARM = True     # PE warmup matmuls
F_NORM = True     # evac-first deferred normalize
F_TRIG = True     # DMA trigger layout keeping exp engines clear
MASK_NEG = -500.0  # zeroes exp in both the ACT and Schraudolph paths

# Schraudolph exp on DVE: int16 i = A*s_raw + B, bits(i) as bf16 ~ exp(s/8).
# For raw scores in +-150 and masked scores near -500 the int result stays
# in [0, 32767], so fp32->int16 conversion semantics (round vs truncate,
# saturate vs wrap) cannot produce a negative or wrapped pattern.
SCH_A = 128.0 / math.log(2.0) / 8.0        # 23.0831...
SCH_B = 127 * 128 - 7.3 + 0.25             # sigma shift + cast-mode split

_NC_CACHE = {}
LAST_RESULTS = None


def _dve_exp_j(p, sb, j, nkt):
    """Which key tiles use the VectorE Schraudolph exp. Avoid the first
    and last couple of each superblock: the z-PSUM evacuation sits in the
    DVE queue right after a superblock boundary, and the evacuation of
    THIS superblock must not queue behind a 1.2us DVE op."""
    if not F_SCH:
        return False
    if j < 3 or j > nkt - 3:
        return False
    if p == 3:
        # none: ScalarE stays a dedicated low-latency exp engine for the
        # last pair, DVE carries the oproj copies
        return False
    return j % 3 == 0


def _build_nc():
    nc = bacc.Bacc("TRN2", target_bir_lowering=False, debug=False)
    xt_d = nc.dram_tensor("xt", [DM, S], BF16, kind="ExternalInput")
    wq_d = nc.dram_tensor("wq", [PAIRS, MC, 128, 128], BF16, kind="ExternalInput")
    wk_d = nc.dram_tensor("wk", [PAIRS, MC, 128, 128], BF16, kind="ExternalInput")
    wv_d = nc.dram_tensor("wv", [MC, 128, NHC * DH], BF16, kind="ExternalInput")
    wo_d = nc.dram_tensor("wo", [PAIRS, 128, DM], BF16, kind="ExternalInput")
    bq_d = nc.dram_tensor("bq", [PAIRS, 128, 1], F32, kind="ExternalInput")
    bk_d = nc.dram_tensor("bk", [PAIRS, 128, 1], F32, kind="ExternalInput")
    msk_d = nc.dram_tensor("mask", [128, 128], BF16, kind="ExternalInput")
    id_d = nc.dram_tensor("ident", [128, 128], BF16, kind="ExternalInput")
    out_d = nc.dram_tensor("out", [S, DM], BF16, kind="ExternalOutput")

    with tile.TileContext(nc) as tc:
      with (
          tc.tile_pool(name="hold", bufs=1) as hold,
          tc.tile_pool(name="ph2", bufs=1) as ph2,
          tc.tile_pool(name="patn", bufs=1, space="PSUM") as patn,
      ):
        v_t = [hold.tile([128, NHC, DH + 1], BF16, tag=f"v{st}", name=f"v{st}") for st in range(NST)]
        z_t = [hold.tile([128, S], BF16, tag=f"z{p}", name=f"z{p}") for p in range(PAIRS)]
        msk_t = hold.tile([128, 128], BF16, tag="mtri")
        id_t = hold.tile([128, 128], BF16, tag="ident")
        bq_t = [hold.tile([128, 1], F32, tag=f"bq{p}", name=f"bq{p}") for p in range(PAIRS)]
        bk_t = [hold.tile([128, 1], F32, tag=f"bk{p}", name=f"bk{p}") for p in range(PAIRS)]
        ones_c = hold.tile([128, 1], F32, tag="ones")
        warm_s = hold.tile([128, 128], BF16, tag="warms")
        warm_m = hold.tile([128, 512], BF16, tag="warmm")
        qts = {}

        nc.vector.memset(ones_c[:], 1.0)
        nc.vector.memset(warm_s[:], 0.0)
        nc.vector.memset(warm_m[:], 0.0)
        # small constants go through the (otherwise idle) gpsimd DMA queue so
        # their triggers don't delay the xt/wv bulk loads
        nc.gpsimd.dma_start(msk_t[:], msk_d.ap())
        nc.gpsimd.dma_start(id_t[:], id_d.ap())
        for p in range(PAIRS):
            nc.gpsimd.dma_start(bq_t[p][:], bq_d.ap()[p])
            nc.gpsimd.dma_start(bk_t[p][:], bk_d.ap()[p])

        def attn_j(p, sb, j, z0, z1):
            qt, kt = qts[p]
            qtb = qt[sb]
            ktb = kt[j // 4]
            nkt = 4 * (sb + 1)
            # columns q < j*128 of this key tile are fully masked;
            # restrict S/exp/PV to the valid suffix.
            j_rel = j - 4 * sb
            off = max(j_rel, 0) * 128
            sp = patn.tile([128, 1024], F32, tag="sp", bufs=2, name="sp")
            ks = ((j % 4) * 128, (j % 4 + 1) * 128)
            diag = j_rel >= 0
            nc.tensor.matmul(
                sp[:, off:512],
                ktb[0:64, ks[0]:ks[1]],
                qtb[0:64, off:SBW],
                start=True, stop=not diag,
                tile_position=(0, 0),
                skip_group_check=True,
            )
            nc.tensor.matmul(
                sp[:, 512 + off:1024],
                ktb[64:128, ks[0]:ks[1]],
                qtb[64:128, off:SBW],
                start=True, stop=not diag,
                tile_position=(64, 0),
                skip_group_check=True,
            )
            if diag:
                # add the causal triangle to the diagonal block in-PSUM:
                # out += I.T @ mask  (PE accumulate, no DVE on critical path)
                for u in (0, 1):
                    lo = u * 512 + off
                    nc.tensor.matmul(
                        sp[:, lo:lo + 128],
                        id_t[:],
                        msk_t[:],
                        start=False, stop=True,
                        skip_group_check=True,
                    )
            pt = ph2.tile([128, 1024], BF16, tag="pt", bufs=6, name="pt")
            sp3 = sp[:].rearrange("p (u q) -> p u q", u=2)
            if _dve_exp_j(p, sb, j, nkt):
                pt3i = pt[:].bitcast(I16).rearrange("p (u q) -> p u q", u=2)
                nc.vector.tensor_scalar(
                    pt3i[:, :, off:512], sp3[:, :, off:512],
                    SCH_A, SCH_B,
                    mybir.AluOpType.mult, mybir.AluOpType.add,
                )
            else:
                pt3 = pt[:].rearrange("p (u q) -> p u q", u=2)
                nc.scalar.activation(
                    pt3[:, :, off:512], sp3[:, :, off:512], EXPF, scale=SCALE
                )
            nc.tensor.matmul(
                z0[:, off:512],
                v_t[j][:, 2 * p, :],
                pt[:, off:512],
                start=(j == 0), stop=(j == nkt - 1),
            )
            nc.tensor.matmul(
                z1[:, off:512],
                v_t[j][:, 2 * p + 1, :],
                pt[:, 512 + off:1024],
                start=(j == 0), stop=(j == nkt - 1),
            )

        pending_norm = []

        def attn_norm_base(p, sb, z0, z1):
            # baseline normalize: d-rows copied from psum, muls read psum
            qs = (sb * SBW, (sb + 1) * SBW)
            d0row = ph2.tile([1, 512], F32, tag="d0row", bufs=2, name="d0row")
            d1row = ph2.tile([1, 512], F32, tag="d1row", bufs=2, name="d1row")
            nc.vector.tensor_copy(d0row[:], z0[DH:DH + 1, :])
            nc.vector.tensor_copy(d1row[:], z1[DH:DH + 1, :])
            nc.vector.reciprocal_approx_fast(d0row[:], d0row[:])
            nc.vector.reciprocal_approx_fast(d1row[:], d1row[:])
            r0 = ph2.tile([64, 512], F32, tag="r0", bufs=2, name="r0")
            r1 = ph2.tile([64, 512], F32, tag="r1", bufs=2, name="r1")
            nc.gpsimd.partition_broadcast(r0[:], d0row[:], channels=64)
            nc.gpsimd.partition_broadcast(r1[:], d1row[:], channels=64)
            nc.vector.tensor_mul(z_t[p][0:64, qs[0]:qs[1]], z0[0:64, :], r0[:])
            t1 = ph2.tile([64, 512], BF16, tag="t1", bufs=2, name="t1")
            nc.vector.tensor_mul(t1[:], z1[0:64, :], r1[:])
            nc.sync.dma_start(z_t[p][64:128, qs[0]:qs[1]], t1[:])

        def attn_norm_a(p, sb, z0, z1):
            if not F_NORM:
                attn_norm_base(p, sb, z0, z1)
                return
            # Evacuate z psum to SBUF first: the bank frees after the two
            # copies (~1us), so the next superblock's PV only waits for
            # them, not the reciprocal/broadcast/multiply chain. The
            # multiplies are deferred (flush_norm) so they never
            # head-of-line block the DVE queue while waiting on the gpsimd
            # broadcast. The denominator row is copied on its own (psum
            # partition 64 -> sbuf partition 0) and recip'd in place:
            # DVE lanes are partition-hardwired, so an SBUF p64->p0 read
            # is not expressible (the psum copy path handles the shift).
            zu0 = ph2.tile([DH, 512], BF16, tag="zu0", bufs=2, name="zu0")
            zu1 = ph2.tile([DH, 512], BF16, tag="zu1", bufs=2, name="zu1")
            d0row = ph2.tile([1, 512], F32, tag="d0row", bufs=2, name="d0row")
            d1row = ph2.tile([1, 512], F32, tag="d1row", bufs=2, name="d1row")
            nc.vector.tensor_copy(d0row[:], z0[DH:DH + 1, :])
            nc.vector.tensor_copy(d1row[:], z1[DH:DH + 1, :])
            nc.vector.tensor_copy(zu0[:], z0[0:DH, :])
            nc.vector.tensor_copy(zu1[:], z1[0:DH, :])
            nc.vector.reciprocal_approx_fast(d0row[:], d0row[:])
            nc.vector.reciprocal_approx_fast(d1row[:], d1row[:])
            r0 = ph2.tile([64, 512], F32, tag="r0", bufs=2, name="r0")
            r1 = ph2.tile([64, 512], F32, tag="r1", bufs=2, name="r1")
            nc.gpsimd.partition_broadcast(r0[:], d0row[:], channels=64)
            nc.gpsimd.partition_broadcast(r1[:], d1row[:], channels=64)
            pending_norm.append((p, sb, zu0, zu1, r0, r1))

        def flush_norm():
            while pending_norm:
                p, sb, zu0, zu1, r0, r1 = pending_norm.pop(0)
                qs = (sb * SBW, (sb + 1) * SBW)
                nc.vector.tensor_mul(z_t[p][0:64, qs[0]:qs[1]], zu0[:], r0[:])
                t1 = ph2.tile([64, 512], BF16, tag="t1", bufs=2, name="t1")
                nc.vector.tensor_mul(t1[:], zu1[:], r1[:])
                nc.sync.dma_start(z_t[p][64:128, qs[0]:qs[1]], t1[:])

        with (
            tc.tile_pool(name="ph1", bufs=1) as ph1,
            tc.tile_pool(name="pqk", bufs=1, space="PSUM") as pqk,
        ):
            # xt in per-512-column-block tiles: attention(0, sb) needs only
            # Q/K columns <= (sb+1)*512 (causal), so the whole front of the
            # kernel pipelines by column block.
            xt_t = [[ph1.tile([128, SBW], BF16, tag=f"xt{m}_{cb}", name=f"xt{m}_{cb}")
                     for cb in range(NSB)] for m in range(MC)]
            wv_t = [ph1.tile([128, NHC * DH], BF16, tag=f"wv{m}", name=f"wv{m}") for m in range(MC)]

            def warm(n):
                if not F_WARM:
                    return
                for _ in range(n):
                    wp = pqk.tile([128, 512], F32, tag="acc", bufs=2, name="warmp")
                    nc.tensor.matmul(wp[:], warm_s[:], warm_m[:],
                                     start=True, stop=True)

            def qk_gen(p, sb_outer=False):
                """QK projection for pair p (bf16, pair-stacked partitions),
                yielded one matmul at a time for interleaving. With
                sb_outer=True the superblock loop is outermost so early
                superblocks finish as soon as their xt column block lands."""
                qt = [hold.tile([128, SBW], BF16, tag=f"qt{i}", bufs=2, name=f"qt{i}")
                      for i in range(NSB)]
                kt = [hold.tile([128, SBW], BF16, tag=f"kt{i}", bufs=2, name=f"kt{i}")
                      for i in range(NSB)]
                qts[p] = (qt, kt)
                wqk = []
                for (w_d, b_t, dst) in ((wq_d, bq_t, qt), (wk_d, bk_t, kt)):
                    wts = []
                    for m in range(MC):
                        w = ph1.tile([128, 128], BF16, tag="w", bufs=16, name="w")
                        nc.sync.dma_start(w[:], w_d.ap()[p, m])
                        wts.append(w)
                    wqk.append((wts, b_t, dst))
                order = (
                    [(sb, wb) for sb in range(NSB) for wb in wqk]
                    if sb_outer else
                    [(sb, wb) for wb in wqk for sb in range(NSB)]
                )
                for sb, (wts, b_t, dst) in order:
                    ps = pqk.tile([128, 512], F32, tag="acc", bufs=2, name="acc")
                    for m in range(MC):
                        nc.tensor.matmul(
                            ps[:],
                            wts[m][:],
                            xt_t[m][sb][:],
                            start=(m == 0),
                            stop=(m == MC - 1),
                        )
                        yield
                    nc.vector.tensor_scalar_add(dst[sb][:], ps[:], b_t[p][:])
                    yield

            # warm the PE HAM clock gate while the first xt block lands
            warm(10)

            # column-block pipelined prelude: per block, land xt columns,
            # then V-projection for its 4 seq tiles and pair 0's QK for it.
            g0 = qk_gen(0, sb_outer=True)
            for cb in range(NSB):
                for m in range(MC):
                    # alternate trigger queues: each dma_start costs ~0.6us
                    # of issuing-engine queue time. ScalarE is free until
                    # the first exp (~10us in), so it carries cb0; later
                    # blocks go to sync+gpsimd (consts are done by then)
                    # to keep the exp engines clear.
                    if cb == 0:
                        # spread the first block over three trigger queues so
                        # the first V-proj matmuls start as soon as possible
                        eng = (nc.scalar, nc.sync, nc.gpsimd)[m % 3]
                    else:
                        eng = nc.gpsimd if m % 2 == 0 else nc.sync
                    eng.dma_start(
                        xt_t[m][cb][:],
                        xt_d.ap()[m * 128:(m + 1) * 128, cb * SBW:(cb + 1) * SBW],
                    )
                    if cb == 0:
                        eng2 = (nc.sync, nc.gpsimd, nc.scalar)[m % 3]
                        eng2.dma_start(wv_t[m][:], wv_d.ap()[m])
                for st in range(4 * cb, 4 * cb + 4):
                    ps = pqk.tile([128, 512], F32, tag="acc", bufs=2, name="acc")
                    for m in range(MC):
                        nc.tensor.matmul(
                            ps[:],
                            xt_t[m][cb][:, (st % 4) * 128:(st % 4 + 1) * 128],
                            wv_t[m][:],
                            start=(m == 0),
                            stop=(m == MC - 1),
                        )
                    nc.vector.tensor_copy(
                        v_t[st][:, :, 0:DH],
                        ps[:].rearrange("p (h d) -> p h d", h=NHC),
                    )
                    nc.vector.tensor_copy(
                        v_t[st][:, :, DH],
                        ones_c[:].to_broadcast((128, NHC)),
                    )
                for _ in range(18):  # one QK column-block (2 proj x (8 mm + copy))
                    try:
                        next(g0)
                    except StopIteration:
                        break
                if cb < NSB - 1:
                    warm(3)  # bridge the DMA gap to the next column block
            for _ in g0:
                pass

            # attention for pairs 0-2, with pair p+1's projection matmuls
            # interleaved into the exp-paced attention stream
            for p in range(3):
                g = qk_gen(p + 1)
                done = False
                emitted = 0
                step = 0
                for sb in range(NSB):
                    nkt = 4 * (sb + 1)
                    z0 = patn.tile([DH + 1, 512], F32, tag="z0", bufs=1, name="z0")
                    z1 = patn.tile([DH + 1, 512], F32, tag="z1", bufs=1, name="z1")
                    for j in range(nkt):
                        attn_j(p, sb, j, z0, z1)
                        if j == 1:
                            flush_norm()
                        step += 1
                        want = 2 * step if step <= 32 else 64 + (step - 32)
                        while emitted < want and not done:
                            try:
                                next(g)
                                emitted += 1
                            except StopIteration:
                                done = True
                    attn_norm_a(p, sb, z0, z1)
                while not done:
                    try:
                        next(g)
                    except StopIteration:
                        done = True

        # ---------------- last pair + output projection ----------------
        with (
            tc.tile_pool(name="ph3", bufs=1) as ph3,
            tc.tile_pool(name="po", bufs=1, space="PSUM") as po,
        ):
            wo_t = [ph3.tile([128, DM], BF16, tag=f"wo{p}", name=f"wo{p}") for p in range(PAIRS)]
            for p in range(PAIRS):
                nc.sync.dma_start(wo_t[p][:], wo_d.ap()[p])

            # output projection, split per group: the pair-0..2 matmuls
            # (oproj_a) only need z that was final before pair 3 started,
            # so they emit freely; the pair-3 matmul + evacuation (oproj_b)
            # emits only once that superblock's z_t[3] normalize chain has
            # had time to complete, so it never head-of-line blocks the PE
            # queue. Groups get their own PSUM banks (the projection pool's
            # two, free in this phase) so scores/PV psum rotation stays
            # decoupled.
            def oproj_a(q, mb, tag="ops"):
                if tag == "ops":
                    ps = po.tile([128, 512], F32, tag="ops", bufs=2, name="ops")
                else:
                    # drain-phase only: attention is over, so the scores/z
                    # psum banks are free for extra in-flight groups
                    ps = patn.tile([128, 512], F32, tag=tag,
                                   bufs=2 if tag == "sp" else 1, name="opsf")
                for p in range(3):
                    nc.tensor.matmul(
                        ps[:],
                        z_t[p][:, q * 128:(q + 1) * 128],
                        wo_t[p][:, mb * 512:(mb + 1) * 512],
                        start=(p == 0),
                        stop=False,
                    )
                return ps

            scnt = [0]

            def oproj_b(q, mb, ps, cp=None, st=None):
                nc.tensor.matmul(
                    ps[:],
                    z_t[3][:, q * 128:(q + 1) * 128],
                    wo_t[3][:, mb * 512:(mb + 1) * 512],
                    start=False,
                    stop=True,
                )
                ost = ph3.tile([128, 512], BF16, tag="ost", bufs=4, name="ost")
                (cp or nc.vector.tensor_copy)(ost[:], ps[:])
                if st is None:
                    st = nc.sync
                st.dma_start(
                    out_d.ap()[q * 128:(q + 1) * 128, mb * 512:(mb + 1) * 512],
                    ost[:],
                )

            otodo = [(q, mb) for q in range(NST) for mb in range(2)]
            na = 0          # groups with oproj_a emitted
            inflight = []   # (q, mb, ps, tag) awaiting oproj_b
            for sb in range(NSB):
                nkt = 4 * (sb + 1)
                z0 = patn.tile([DH + 1, 512], F32, tag="z0", bufs=1, name="z0")
                z1 = patn.tile([DH + 1, 512], F32, tag="z1", bufs=1, name="z1")
                for j in range(nkt):
                    attn_j(3, sb, j, z0, z1)
                    if j == 0:
                        # z_t[3] of superblock sb-1 is written by these muls
                        flush_norm()
                    # b: z_t[3] of superblock s is ready ~5 key tiles into
                    # superblock s+1 (evac+recip+broadcast+mul+dma chain);
                    # for s < sb-1 it has long been final, emit any time
                    for _ in range(2):
                        if not inflight:
                            break
                        s_old = inflight[0][0] // 4
                        if s_old < sb - 1 or (j >= 5 and s_old < sb):
                            q, mb, ps, _t = inflight.pop(0)
                            oproj_b(q, mb, ps)
                        else:
                            break
                    if len(inflight) < 2 and na < len(otodo):
                        q, mb = otodo[na]
                        inflight.append((q, mb, oproj_a(q, mb), "ops"))
                        na += 1
                attn_norm_a(3, sb, z0, z1)
            flush_norm()
            # drain: all z_t[3] is final once the muls above ran. Deepen
            # in-flight with the scores/z psum banks (attention is over) so
            # the pair-0..2 matmuls of every remaining group run while the
            # final normalize chain completes; copies alternate
            # scalar/vector (exp is done, ScalarE is free now).
            free_tags = ["sp", "sp", "z0", "z1"] + ["ops"] * (2 - len(inflight))
            ocnt = 0
            while inflight or na < len(otodo):
                while na < len(otodo) and free_tags:
                    t = free_tags.pop(0)
                    q, mb = otodo[na]
                    inflight.append((q, mb, oproj_a(q, mb, t), t))
                    na += 1
                q, mb, ps, t = inflight.pop(0)
                cp = nc.scalar.copy if ocnt % 2 == 0 else nc.vector.tensor_copy
                # alternate store triggers too: the sync queue serializes at
                # ~0.7us per trigger, which otherwise stretches the tail
                st = nc.gpsimd if ocnt % 2 == 0 else nc.sync
                ocnt += 1
                oproj_b(q, mb, ps, cp=cp, st=st)
                free_tags.append(t)

    nc.compile()
    return nc


def _get_nc():
    if "nc" not in _NC_CACHE:
        _NC_CACHE["nc"] = _build_nc()
    return _NC_CACHE["nc"]


def _causal_masks():
    k = np.arange(128)[:, None]
    q = np.arange(128)[None, :]
    return np.where(q >= k, 0.0, MASK_NEG).astype(ml_dtypes.bfloat16)


def kernel(resid_pre, W_Q, W_K, W_V, W_O, b_Q, b_K, b_V, b_O):
    global LAST_RESULTS
    resid_pre = np.asarray(resid_pre, dtype=np.float32)
    W_Q = np.asarray(W_Q, dtype=np.float32)
    W_K = np.asarray(W_K, dtype=np.float32)
    W_V = np.asarray(W_V, dtype=np.float32)
    W_O = np.asarray(W_O, dtype=np.float32)
    b_Q = np.asarray(b_Q, dtype=np.float32)
    b_K = np.asarray(b_K, dtype=np.float32)
    b_V = np.asarray(b_V, dtype=np.float32)
    b_O = np.asarray(b_O, dtype=np.float32)

    B = resid_pre.shape[0]
    masks = _causal_masks()
    ident = np.eye(128, dtype=ml_dtypes.bfloat16)

    def pack_pairs(w):  # [8, 1024, 64] -> [4, 8, 128, 128]
        return np.ascontiguousarray(
            w.reshape(PAIRS, 2, DM, DH).transpose(0, 2, 1, 3).reshape(PAIRS, MC, 128, 128)
        )

    in_maps = []
    for c in range(8):
        b, g = divmod(c, 2)
        hs = slice(g * NHC, (g + 1) * NHC)
        in_maps.append({
            "xt": np.ascontiguousarray(resid_pre[b].T).astype(ml_dtypes.bfloat16),
            "wq": pack_pairs(W_Q[hs]).astype(ml_dtypes.bfloat16),
            "wk": pack_pairs(W_K[hs]).astype(ml_dtypes.bfloat16),
            "wv": np.ascontiguousarray(
                W_V[hs].transpose(1, 0, 2).reshape(DM, NHC * DH).reshape(MC, 128, NHC * DH)
            ).astype(ml_dtypes.bfloat16),
            "wo": np.ascontiguousarray(W_O[hs].reshape(PAIRS, 128, DM)).astype(ml_dtypes.bfloat16),
            "bq": np.ascontiguousarray(b_Q[hs].reshape(PAIRS, 128, 1)),
            "bk": np.ascontiguousarray(b_K[hs].reshape(PAIRS, 128, 1)),
            "mask": masks,
            "ident": ident,
        })

    nc = _get_nc()
    res = bass_utils.run_bass_kernel_spmd(nc, in_maps, core_ids=list(range(8)))
    LAST_RESULTS = res

    # b_V contributes exactly sum_h W_O[h].T @ b_V[h] (softmax rows sum to 1)
    const = np.einsum("hdm,hd->m", W_O, b_V).astype(np.float32) + b_O
    out = np.empty((B, S, DM), dtype=np.float32)
    for b in range(B):
        out[b] = (res.results[2 * b]["out"].astype(np.float32)
                  + res.results[2 * b + 1]["out"].astype(np.float32) + const)
    return out
